# revision 1
# baseline (speedup 1.0000x reference)
"""Self-contained Trainium2 Bass kernel for the CharRNN problem:
2-layer LSTM (B=32, T=256, H=256) + V=32000 softmax cross-entropy mean loss.

Strategy (8 NeuronCores, SPMD):
  * the LSTM recurrence is replicated on every core (it is latency-bound, so
    batch-sharding would not make it faster and would need collectives)
  * the dominant softmax matmul + exp is sharded over the vocab: each core
    owns a 4000-wide shard of softmax_w, computes logits for all 8192 rows
    against its shard, and reduces them to per-row sum(exp(logit)) plus the
    per-row target logit (rows whose target falls in the shard)
  * the host combines: loss_r = log(sum_cores se_r) - tgt_logit_r

Device-side layout (per core):
  * rows are TIME-MAJOR: r = t*B + b, so a 128-row tile = 4 timesteps
  * xs^T / hs^T activation slabs [128, 8192] bf16, hidden dim on partitions
  * LSTM gates z: psum [32, 1024] (batch on partitions), gate columns
    permuted to [i, o, f, j] on the host so one sigmoid covers [i, o]
  * h is transposed back to hidden-major each step with PE-transposes
  * target logits: gather softmax_w columns by target id (gpsimd ap_gather
    over an int16-pair view), multiply with hs^T, reduce with a ones-vector
    matmul
"""
import os
import numpy as np
import ml_dtypes
import concourse.bass as bass
import concourse.mybir as mybir
import concourse.tile as tile
from concourse import bacc
from concourse.masks import make_identity
from concourse.bass_utils import run_bass_kernel_spmd

F32 = mybir.dt.float32
BF16 = mybir.dt.bfloat16
I32 = mybir.dt.int32
I16 = mybir.dt.int16
AF = mybir.ActivationFunctionType
ALU = mybir.AluOpType

B, T, H, V, NCORES = 32, 256, 256, 32000, 8


def build_charrnn(T=256, V=32000, n_cores=8, has_b1=False, has_b2=False,
                  has_swb=False, num_devices=8):
    B, H = 32, 256
    G4 = 4 * H                      # 1024 gate width
    VS = V // n_cores               # vocab shard per core
    BT = B * T
    RT = BT // 128                  # 128-row tiles (4 steps each)
    assert T % 4 == 0 and BT % 128 == 0

    # vocab chunking for the exp pass: one psum BANK per chunk — a matmul
    # may not cross a psum bank boundary (HW corrupts accumulation if the
    # write spans banks; sim does not model this)
    CH = max(d for d in range(1, 513) if VS % d == 0)
    NCHUNK = VS // CH

    nc = bacc.Bacc("TRN2", target_bir_lowering=False, debug=False,
                   num_devices=num_devices)

    # ---------------- DRAM I/O ----------------
    ids_d = nc.dram_tensor("ids", (RT, 128, 1), I32, kind="ExternalInput")
    emb_d = nc.dram_tensor("emb", (V, H), F32, kind="ExternalInput")
    w1_d = nc.dram_tensor("w1", (4, 128, G4), BF16, kind="ExternalInput")
    w2_d = nc.dram_tensor("w2", (4, 128, G4), BF16, kind="ExternalInput")
    sw_d = nc.dram_tensor("sw", (2, 128, VS), BF16, kind="ExternalInput")
    swp_d = nc.dram_tensor("swp", (2, 128, VS, 2), I16, kind="ExternalInput")
    tgi_d = nc.dram_tensor("tgi", (RT, 128, 8), I16, kind="ExternalInput")
    if has_b1:
        b1_d = nc.dram_tensor("b1p", (32, G4), F32, kind="ExternalInput")
    if has_b2:
        b2_d = nc.dram_tensor("b2p", (32, G4), F32, kind="ExternalInput")
    if has_swb:
        swb_d = nc.dram_tensor("swbp", (128, VS), F32, kind="ExternalInput")
    se_d = nc.dram_tensor("se_out", (128, RT * NCHUNK), F32,
                          kind="ExternalOutput")
    tg_d = nc.dram_tensor("tg_out", (1, BT), F32, kind="ExternalOutput")

    with tile.TileContext(nc) as tc:
        with tc.tile_pool(name="persist", bufs=1) as pp:
            # ---- persistent SBUF ----
            w1_sb = pp.tile([128, 4, G4], BF16, tag="w1")
            w2_sb = pp.tile([128, 4, G4], BF16, tag="w2")
            nc.sync.dma_start(w1_sb[:], w1_d[:].rearrange("k p c -> p k c"))
            nc.sync.dma_start(w2_sb[:], w2_d[:].rearrange("k p c -> p k c"))
            sw_sb = pp.tile([128, 2, VS], BF16, tag="sw")
            nc.sync.dma_start(sw_sb[:], sw_d[:].rearrange("k p c -> p k c"))
            swp_sb = pp.tile([128, 2, VS, 2], I16, tag="swp")
            nc.sync.dma_start(swp_sb[:],
                              swp_d[:].rearrange("k p c d -> p k c d"))
            hs0 = pp.tile([128, BT], BF16, tag="hs0")
            hs1 = pp.tile([128, BT], BF16, tag="hs1")

            ones_bf = pp.tile([128, 1], BF16, tag="ones")
            nc.gpsimd.memset(ones_bf[:], 1.0)
            half_sb = pp.tile([128, 1], F32, tag="half")
            nc.gpsimd.memset(half_sb[:], 0.5)

            c1 = pp.tile([32, H], F32, tag="c1")
            c2 = pp.tile([32, H], F32, tag="c2")
            nc.gpsimd.memset(c1[:], 0.0)
            nc.gpsimd.memset(c2[:], 0.0)

            se_sb = pp.tile([128, RT * NCHUNK], F32, tag="se")
            tg_sb = pp.tile([1, BT], F32, tag="tg")
            # accum_out adds into existing SBUF content on HW — zero it
            nc.gpsimd.memset(se_sb[:], 0.0)

            if has_b1:
                b1_sb = pp.tile([32, G4], F32, tag="b1")
                nc.sync.dma_start(b1_sb[:], b1_d[:])
            if has_b2:
                b2_sb = pp.tile([32, G4], F32, tag="b2")
                nc.sync.dma_start(b2_sb[:], b2_d[:])
            if has_swb:
                swb_sb = pp.tile([128, VS], F32, tag="swb")
                nc.sync.dma_start(swb_sb[:], swb_d[:])

            # ============ fused phase: gather + LSTM + logits ============
            with (
                tc.tile_pool(name="xsp", bufs=1) as xsp,
                tc.tile_pool(name="stage", bufs=3) as stp,
                tc.tile_pool(name="lwork", bufs=3) as lw,
                tc.tile_pool(name="zp", bufs=2, space="PSUM") as zp,
                tc.tile_pool(name="ep", bufs=3, space="PSUM") as ep,
                tc.tile_pool(name="ework", bufs=3) as ew,
            ):
                xs0 = xsp.tile([128, BT], BF16, tag="xs0")
                xs1 = xsp.tile([128, BT], BF16, tag="xs1")

                # ---- embedding gather (time-major) + transpose to slabs ----
                for rt in range(RT):
                    ids_sb = stp.tile([128, 1], I32, tag="ids")
                    nc.gpsimd.dma_start(ids_sb[:], ids_d.ap()[rt])
                    xrow = stp.tile([128, H], F32, tag="xrow")
                    nc.gpsimd.indirect_dma_start(
                        out=xrow[:], out_offset=None,
                        in_=emb_d[:],
                        in_offset=bass.IndirectOffsetOnAxis(
                            ap=ids_sb[:, :1], axis=0),
                    )
                    xbf = stp.tile([128, H], BF16, tag="xbf")
                    nc.vector.tensor_copy(xbf[:], xrow[:])
                    cs = 128 * rt
                    nc.sync.dma_start_transpose(
                        xs0[:, cs:cs + 128], xbf[:, 0:128])
                    nc.sync.dma_start_transpose(
                        xs1[:, cs:cs + 128], xbf[:, 128:256])

                def emit_logits_tile(rt):
                    cs = 128 * rt
                    for c0 in range(0, NCHUNK, 2):
                        cpair = [c for c in (c0, c0 + 1) if c < NCHUNK]
                        pses = []
                        for _c in cpair:
                            pse_t = ep.tile([128, CH], F32, tag="pse")
                            pses.append(pse_t)
                        for k in range(2):
                            hsk = hs0[:, cs:cs + 128] if k == 0 \
                                else hs1[:, cs:cs + 128]
                            for pse, c in zip(pses, cpair):
                                nc.tensor.matmul(
                                    pse[:], hsk,
                                    sw_sb[:, k, c * CH:c * CH + CH],
                                    start=(k == 0), stop=(k == 1),
                                )
                        for pse, c in zip(pses, cpair):
                            if has_swb:
                                nc.vector.tensor_tensor(
                                    out=pse[:], in0=pse[:],
                                    in1=swb_sb[:, c * CH:c * CH + CH],
                                    op=ALU.add)
                            ebuf = ew.tile([128, CH], BF16, tag="ebuf")
                            nc.scalar.activation(
                                ebuf[:], pse[:], AF.Exp,
                                accum_out=se_sb[:, rt * NCHUNK + c:
                                                rt * NCHUNK + c + 1])
                    # target logit for these 128 rows
                    tgi_sb = ew.tile([128, 8], I16, tag="tgi")
                    nc.gpsimd.dma_start(tgi_sb[:], tgi_d.ap()[rt])
                    pst = ep.tile([1, 128], F32, tag="pse")
                    for k in range(2):
                        swg = ew.tile([128, 128, 2], I16, tag="swg")
                        nc.gpsimd.ap_gather(
                            swg[:], swp_sb[:, k], tgi_sb[:],
                            channels=128, num_elems=VS, d=2, num_idxs=128,
                        )
                        mulk = ew.tile([128, 128], BF16, tag="mulk")
                        nc.vector.tensor_tensor(
                            out=mulk[:],
                            in0=swg[:].bitcast(BF16)[:, :, 0],
                            in1=hs0[:, cs:cs + 128] if k == 0
                            else hs1[:, cs:cs + 128],
                            op=ALU.mult)
                        nc.tensor.matmul(pst[:], ones_bf[:, 0:1], mulk[:],
                                         start=(k == 0), stop=(k == 1))
                    nc.scalar.copy(tg_sb[0:1, cs:cs + 128], pst[:])

                # ---- LSTM over T steps ----
                h1T_prev = None  # [128, 64] bf16 (k-tiles of h1^T)
                for t in range(T):
                    ts0 = 32 * t

                    def lstm_layer(lhsTs, w_sb, c_sb, bias_sb):
                        """One LSTM layer step. lhsTs: list of [128,32] bf16
                        k-tiles. Returns h_row [32, 256] bf16."""
                        psz = zp.tile([32, G4], F32, tag="z")
                        nk = len(lhsTs)
                        for k, lt in enumerate(lhsTs):
                            for nh in range(2):
                                nc.tensor.matmul(
                                    psz[:, 512 * nh:512 * nh + 512],
                                    lt,
                                    w_sb[:, k, 512 * nh:512 * nh + 512],
                                    start=(k == 0), stop=(k == nk - 1),
                                )
                        if bias_sb is not None:
                            nc.vector.tensor_tensor(
                                out=psz[:], in0=psz[:],
                                in1=bias_sb[:],
                                op=ALU.add)
                        # gates (host col order): i[0:256] o[256:512]
                        # f[512:768] j[768:1024]. sigmoid(x) is computed as
                        # 0.5*tanh(x/2)+0.5 (tanh+exp share one ACT table
                        # set, so LSTM and softmax-exp can interleave); the
                        # 0.5/0.5 affine folds into affine_mul_reduce.
                        g = lw.tile([32, G4], BF16, tag="g")
                        nc.scalar.activation(g[:, 0:512], psz[:, 0:512],
                                             AF.Tanh, scale=0.5)
                        nc.scalar.activation(g[:, 512:768], psz[:, 512:768],
                                             AF.Tanh, bias=half_sb[0:32, :1],
                                             scale=0.5)
                        nc.scalar.activation(g[:, 768:1024], psz[:, 768:1024],
                                             AF.Tanh)
                        # c = sig(f+1)*c + sig(i)*j ; h = tanh(c)*sig(o)
                        junk = lw.tile([32, 1], F32, tag="junk")
                        t1 = lw.tile([32, H], BF16, tag="t1")
                        nc.vector.affine_mul_reduce(
                            t1[:], junk[:], g[:, 0:256], g[:, 768:1024],
                            0.5, 0.5)
                        cf = lw.tile([32, H], F32, tag="cf")
                        nc.vector.affine_mul_reduce(
                            cf[:], junk[:], g[:, 512:768], c_sb[:], 0.5, 0.5)
                        nc.vector.tensor_tensor(out=c_sb[:], in0=cf[:],
                                                in1=t1[:], op=ALU.add)
                        tc_t = lw.tile([32, H], BF16, tag="tc")
                        nc.scalar.activation(tc_t[:], c_sb[:], AF.Tanh)
                        hrow = lw.tile([32, H], BF16, tag="hrow")
                        nc.vector.affine_mul_reduce(
                            hrow[:], junk[:], g[:, 256:512], tc_t[:],
                            0.5, 0.5)
                        return hrow

                    # layer 1: x k-tiles + h1 k-tiles
                    lhsTs = [xs0[:, ts0:ts0 + 32], xs1[:, ts0:ts0 + 32]]
                    if h1T_prev is not None:
                        lhsTs += [h1T_prev[:, 0:32], h1T_prev[:, 32:64]]
                    h1row = lstm_layer(lhsTs, w1_sb, c1,
                                       b1_sb if has_b1 else None)
                    # transpose h1 -> hidden-major k-tiles via the DMA xbar
                    # (keeps TensorE free)
                    h1T = lw.tile([128, 64], BF16, tag="h1T")
                    nc.sync.dma_start_transpose(h1T[:, 0:32], h1row[:, 0:128])
                    nc.sync.dma_start_transpose(h1T[:, 32:64],
                                                h1row[:, 128:256])
                    h1T_prev = h1T

                    # layer 2: h1 k-tiles + h2 k-tiles (prev step)
                    lhsTs = [h1T[:, 0:32], h1T[:, 32:64]]
                    if t > 0:
                        tp = 32 * (t - 1)
                        lhsTs += [hs0[:, tp:tp + 32], hs1[:, tp:tp + 32]]
                    h2row = lstm_layer(lhsTs, w2_sb, c2,
                                       b2_sb if has_b2 else None)
                    nc.sync.dma_start_transpose(hs0[:, ts0:ts0 + 32],
                                                h2row[:, 0:128])
                    nc.sync.dma_start_transpose(hs1[:, ts0:ts0 + 32],
                                                h2row[:, 128:256])

                    # interleave the logits/softmax tile for rows that just
                    # completed (keeps TensorE dense so HAM stays warm)
                    if t % 4 == 3:
                        emit_logits_tile(t // 4)

            nc.sync.dma_start(se_d[:], se_sb[:])
            nc.sync.dma_start(tg_d[:], tg_sb[:])

    nc.compile()
    meta = dict(T=T, V=V, n_cores=n_cores, B=B, H=H, VS=VS, BT=BT, RT=RT,
                CH=CH, NCHUNK=NCHUNK)
    return nc, meta


# ---------------- host-side prep / combine ----------------

def prep_inputs(meta, input_data, targets, embedding, W1, b1, W2, b2,
                softmax_w, softmax_b):
    """Build the per-core input maps (numpy)."""
    B, T, V = meta["B"], meta["T"], meta["V"]
    VS, RT, n_cores = meta["VS"], meta["RT"], meta["n_cores"]
    H = meta["H"]
    G4 = 4 * H

    ids_tm = np.ascontiguousarray(
        np.asarray(input_data, np.int64).T).reshape(-1)
    tgt_tm = np.ascontiguousarray(
        np.asarray(targets, np.int64).T).reshape(-1)
    ids_in = ids_tm.astype(np.int32).reshape(RT, 128, 1)

    # W column permutation [i, j, f, o] (TF order) -> [i, o, f, j]
    perm = np.concatenate([
        np.arange(0, H), np.arange(3 * H, 4 * H),
        np.arange(2 * H, 3 * H), np.arange(H, 2 * H)])

    def prep_w(W):
        Wp = W[:, perm].astype(ml_dtypes.bfloat16)          # [512, 1024]
        return np.ascontiguousarray(Wp.reshape(4, 128, G4))

    w1_in = prep_w(np.asarray(W1, np.float32))
    w2_in = prep_w(np.asarray(W2, np.float32))
    b1p = np.tile(np.asarray(b1, np.float32)[perm].reshape(1, G4), (32, 1))
    b2p = np.tile(np.asarray(b2, np.float32)[perm].reshape(1, G4), (32, 1))

    sw = np.asarray(softmax_w, np.float32)                  # [H, V]
    swb = np.asarray(softmax_b, np.float32)

    # vectorized ap_gather index layout: idx i lives at partition i%16,
    # column i//16, replicated per 16-partition group
    rtA = (np.arange(RT) * 128)[:, None, None]
    pA = (np.arange(128) % 16)[None, :, None]
    qA = (np.arange(8) * 16)[None, None, :]
    gat = rtA + qA + pA                                     # [RT, 128, 8]

    maps, masks = [], []
    for c in range(n_cores):
        shard = sw[:, c * VS:(c + 1) * VS].astype(ml_dtypes.bfloat16)
        sw_in = np.ascontiguousarray(shard.reshape(2, 128, VS))
        swi = sw_in.view(np.int16)
        swp_in = np.ascontiguousarray(
            np.stack([swi, swi], axis=-1))                  # [2,128,VS,2]

        tl = tgt_tm - c * VS
        inr = (tl >= 0) & (tl < VS)
        tlc = np.where(inr, tl, 0).astype(np.int16)
        tgi = tlc[gat]                                      # [RT, 128, 8]
        m = dict(ids=ids_in, emb=np.asarray(embedding, np.float32),
                 w1=w1_in, w2=w2_in, sw=sw_in, swp=swp_in, tgi=tgi)
        if np.any(b1p):
            m["b1p"] = b1p
        if np.any(b2p):
            m["b2p"] = b2p
        if np.any(swb):
            m["swbp"] = np.ascontiguousarray(
                np.tile(swb[c * VS:(c + 1) * VS].reshape(1, VS), (128, 1)))
        maps.append(m)
        masks.append(inr.astype(np.float32))
    return maps, masks, ids_tm, tgt_tm


def combine_outputs(meta, results, masks, tgt_tm, softmax_b):
    """results: list of per-core dicts with se_out [128, RT*NCHUNK] and
    tg_out [1, BT]. Returns the scalar cost (np.float32)."""
    B, T, BT = meta["B"], meta["T"], meta["BT"]
    RT, NCHUNK = meta["RT"], meta["NCHUNK"]
    se_all = np.zeros(BT, np.float64)
    tg_all = np.zeros(BT, np.float64)
    for c, r in enumerate(results):
        se = np.asarray(r["se_out"], np.float64)  # [128, RT*NCHUNK]
        se = se.reshape(128, RT, NCHUNK).sum(-1)  # [128, RT]
        se_all += se.T.reshape(-1)                # row r = rt*128 + p
        tg_all += np.asarray(r["tg_out"], np.float64)[0] * masks[c]
    tg_all += np.asarray(softmax_b, np.float64)[tgt_tm]
    loss = np.log(se_all) - tg_all
    return np.float32(loss.sum() / B / T)


# ---------------- public entry point ----------------

_CACHE = {}
last_exec_time_ns = None
last_trace_path = None


def _get_built(has_b1, has_b2, has_swb):
    key = (has_b1, has_b2, has_swb)
    if key not in _CACHE:
        _CACHE[key] = build_charrnn(T=T, V=V, n_cores=NCORES,
                                    has_b1=has_b1, has_b2=has_b2,
                                    has_swb=has_swb, num_devices=NCORES)
    return _CACHE[key]


def kernel(input_data, targets, embedding, W1, b1, W2, b2,
           softmax_w, softmax_b, _trace=False):
    global last_exec_time_ns, last_trace_path
    has_b1 = bool(np.any(np.asarray(b1)))
    has_b2 = bool(np.any(np.asarray(b2)))
    has_swb = bool(np.any(np.asarray(softmax_b)))
    nc, meta = _get_built(has_b1, has_b2, has_swb)
    maps, masks, ids_tm, tgt_tm = prep_inputs(
        meta, input_data, targets, embedding, W1, b1, W2, b2,
        softmax_w, softmax_b)
    res = run_bass_kernel_spmd(nc, maps, core_ids=list(range(NCORES)),
                               trace=_trace)
    last_exec_time_ns = res.exec_time_ns
    if res.instructions_and_trace is not None:
        last_trace_path = res.instructions_and_trace[1]
    cost = combine_outputs(meta, res.results, masks, tgt_tm, softmax_b)
    return np.asarray(cost, np.float32)



# revision 4
# speedup vs baseline: 1.9933x; 1.9933x over previous
"""Self-contained Trainium2 Bass kernel for the CharRNN problem:
2-layer LSTM (B=32, T=256, H=256) + V=32000 softmax cross-entropy mean loss.

Strategy (8 NeuronCores, SPMD):
  * the LSTM recurrence is replicated on every core (latency-bound)
  * the softmax matmul + exp is sharded over the vocab: each core owns a
    4000-wide shard of softmax_w, computes logits for all 8192 rows against
    its shard, reduces them to per-row sum(exp(logit)) plus the per-row
    target logit; the host combines loss_r = log(sum_c se_r) - tgt_logit_r

Device-side structure (v2 — chain-optimized vs the DMA-transpose baseline):
  * wavefront: slot t runs L1 step t and L2 step t-1 so the two layer
    recurrence chains interleave on the engines
  * h transposes via DVE 32x32 StreamTranspose blocks (cross-partition
    writes straight into the hidden-major slabs) instead of the Sync-queue
    DMA transposes (1.2us each) of the baseline
  * gate column order [i, o, j, f] with the 0.5 sigmoid input scale folded
    into W on the host -> 2 ACT calls per layer-step (tanh over [i,o,j],
    tanh+0.5bias over [f]) instead of 3
  * exp over PAIRS of 500-wide vocab chunks ([128,2,500] strided AP over a
    2-bank psum tile) -> half the EXP/accum instruction count
  * one logits chunk-pair emitted per slot BEFORE the LSTM matmuls: the PE
    has dense fill work while the recurrence chain runs, keeping HAM warm
"""
import numpy as np
import ml_dtypes
import concourse.bass as bass
import concourse.mybir as mybir
import concourse.tile as tile
from concourse import bacc
from concourse.bass_utils import run_bass_kernel_spmd

F32 = mybir.dt.float32
BF16 = mybir.dt.bfloat16
I32 = mybir.dt.int32
I16 = mybir.dt.int16
AF = mybir.ActivationFunctionType
ALU = mybir.AluOpType

B, T, H, V, NCORES = 32, 256, 256, 32000, 8


def build_charrnn(T=256, V=32000, n_cores=8, has_b1=False, has_b2=False,
                  has_swb=False, num_devices=8):
    B, H = 32, 256
    G4 = 4 * H                      # 1024 gate width
    VS = V // n_cores               # vocab shard per core
    BT = B * T
    RT = BT // 128                  # 128-row tiles (4 steps each)
    assert T % 4 == 0 and BT % 128 == 0

    # one psum BANK per matmul chunk (a matmul may not cross a bank)
    CH = max(d for d in range(1, 513) if VS % d == 0)   # 500
    NCHUNK = VS // CH                                    # 8
    NPAIR = NCHUNK // 2                                  # 4 exp calls per tile

    nc = bacc.Bacc("TRN2", target_bir_lowering=False, debug=False,
                   num_devices=num_devices)

    # ---------------- DRAM I/O ----------------
    ids_d = nc.dram_tensor("ids", (RT, 128, 1), I32, kind="ExternalInput")
    emb_d = nc.dram_tensor("emb", (V, H), F32, kind="ExternalInput")
    w1_d = nc.dram_tensor("w1", (4, 128, G4), BF16, kind="ExternalInput")
    w2_d = nc.dram_tensor("w2", (4, 128, G4), BF16, kind="ExternalInput")
    sw_d = nc.dram_tensor("sw", (2, 128, VS), BF16, kind="ExternalInput")
    swp_d = nc.dram_tensor("swp", (2, 128, VS, 2), I16, kind="ExternalInput")
    tgi_d = nc.dram_tensor("tgi", (RT, 128, 8), I16, kind="ExternalInput")
    if has_b1:
        b1_d = nc.dram_tensor("b1p", (32, G4), F32, kind="ExternalInput")
    if has_b2:
        b2_d = nc.dram_tensor("b2p", (32, G4), F32, kind="ExternalInput")
    if has_swb:
        swb_d = nc.dram_tensor("swbp", (128, VS), F32, kind="ExternalInput")
    se_d = nc.dram_tensor("se_out", (128, RT * NPAIR), F32,
                          kind="ExternalOutput")
    tg_d = nc.dram_tensor("tg_out", (1, BT), F32, kind="ExternalOutput")

    with tile.TileContext(nc) as tc:
        with tc.tile_pool(name="persist", bufs=1) as pp:
            # ---- persistent SBUF ----
            w1_sb = pp.tile([128, 4, G4], BF16, tag="w1")
            w2_sb = pp.tile([128, 4, G4], BF16, tag="w2")
            nc.sync.dma_start(w1_sb[:], w1_d[:].rearrange("k p c -> p k c"))
            nc.sync.dma_start(w2_sb[:], w2_d[:].rearrange("k p c -> p k c"))
            sw_sb = pp.tile([128, 2, VS], BF16, tag="sw")
            nc.sync.dma_start(sw_sb[:], sw_d[:].rearrange("k p c -> p k c"))
            swp_sb = pp.tile([128, 2, VS, 2], I16, tag="swp")
            nc.sync.dma_start(swp_sb[:],
                              swp_d[:].rearrange("k p c d -> p k c d"))
            hs0 = pp.tile([128, BT], BF16, tag="hs0")
            hs1 = pp.tile([128, BT], BF16, tag="hs1")

            ones_bf = pp.tile([128, 1], BF16, tag="ones")
            nc.gpsimd.memset(ones_bf[:], 1.0)
            half_sb = pp.tile([128, 1], F32, tag="half")
            nc.gpsimd.memset(half_sb[:], 0.5)

            c1 = pp.tile([32, H], F32, tag="c1")
            c2 = pp.tile([32, H], F32, tag="c2")
            nc.gpsimd.memset(c1[:], 0.0)
            nc.gpsimd.memset(c2[:], 0.0)

            se_sb = pp.tile([128, RT * NPAIR], F32, tag="se")
            tg_sb = pp.tile([1, BT], F32, tag="tg")
            # accum_out adds into existing SBUF content on HW — zero it
            nc.gpsimd.memset(se_sb[:], 0.0)

            if has_b1:
                b1_sb = pp.tile([32, G4], F32, tag="b1")
                nc.sync.dma_start(b1_sb[:], b1_d[:])
            if has_b2:
                b2_sb = pp.tile([32, G4], F32, tag="b2")
                nc.sync.dma_start(b2_sb[:], b2_d[:])
            if has_swb:
                swb_sb = pp.tile([128, VS], F32, tag="swb")
                nc.sync.dma_start(swb_sb[:], swb_d[:])

            # ============ fused phase: gather + LSTM + logits ============
            with (
                tc.tile_pool(name="xsp", bufs=1) as xsp,
                tc.tile_pool(name="stage", bufs=3) as stp,
                tc.tile_pool(name="lwork", bufs=3) as lw,
                tc.tile_pool(name="zp", bufs=2, space="PSUM") as zp,
                tc.tile_pool(name="ep", bufs=2, space="PSUM") as ep,
                tc.tile_pool(name="ework", bufs=3) as ew,
            ):
                xs0 = xsp.tile([128, BT], BF16, tag="xs0")
                xs1 = xsp.tile([128, BT], BF16, tag="xs1")

                # ---- embedding gather (time-major) + transpose to slabs ----
                for rt in range(RT):
                    ids_sb = stp.tile([128, 1], I32, tag="ids")
                    nc.gpsimd.dma_start(ids_sb[:], ids_d.ap()[rt])
                    xrow = stp.tile([128, H], F32, tag="xrow")
                    nc.gpsimd.indirect_dma_start(
                        out=xrow[:], out_offset=None,
                        in_=emb_d[:],
                        in_offset=bass.IndirectOffsetOnAxis(
                            ap=ids_sb[:, :1], axis=0),
                    )
                    xbf = stp.tile([128, H], BF16, tag="xbf")
                    nc.vector.tensor_copy(xbf[:], xrow[:])
                    cs = 128 * rt
                    nc.sync.dma_start_transpose(
                        xs0[:, cs:cs + 128], xbf[:, 0:128])
                    nc.sync.dma_start_transpose(
                        xs1[:, cs:cs + 128], xbf[:, 128:256])

                def emit_logits_pair(rt, p):
                    """Logits+exp for vocab chunks (2p, 2p+1) of row-tile rt;
                    p==3 also emits the target-logit gather for the tile."""
                    cs = 128 * rt
                    pse = ep.tile([128, 2, 512], F32, tag="pse")
                    for half, c in enumerate((2 * p, 2 * p + 1)):
                        for k in range(2):
                            hsk = hs0[:, cs:cs + 128] if k == 0 \
                                else hs1[:, cs:cs + 128]
                            nc.tensor.matmul(
                                pse[:, half, 0:CH], hsk,
                                sw_sb[:, k, c * CH:c * CH + CH],
                                start=(k == 0), stop=(k == 1),
                            )
                        if has_swb:
                            nc.vector.tensor_tensor(
                                out=pse[:, half, 0:CH], in0=pse[:, half, 0:CH],
                                in1=swb_sb[:, (2 * p + half) * CH:
                                           (2 * p + half) * CH + CH],
                                op=ALU.add)
                    ebuf = ew.tile([128, 2, CH], BF16, tag="ebuf")
                    nc.scalar.activation(
                        ebuf[:], pse[:, :, 0:CH], AF.Exp,
                        accum_out=se_sb[:, rt * NPAIR + p:rt * NPAIR + p + 1])
                    if p != 3:
                        return
                    # target logit for these 128 rows
                    tgi_sb = ew.tile([128, 8], I16, tag="tgi")
                    nc.gpsimd.dma_start(tgi_sb[:], tgi_d.ap()[rt])
                    pstt = ep.tile([128, 2, 512], F32, tag="pse")
                    pst = pstt[0:1, 0, 0:128]
                    for k in range(2):
                        swg = ew.tile([128, 128, 2], I16, tag="swg")
                        nc.gpsimd.ap_gather(
                            swg[:], swp_sb[:, k], tgi_sb[:],
                            channels=128, num_elems=VS, d=2, num_idxs=128,
                        )
                        mulk = ew.tile([128, 128], BF16, tag="mulk")
                        nc.vector.tensor_tensor(
                            out=mulk[:],
                            in0=swg[:].bitcast(BF16)[:, :, 0],
                            in1=hs0[:, cs:cs + 128] if k == 0
                            else hs1[:, cs:cs + 128],
                            op=ALU.mult)
                        nc.tensor.matmul(pst, ones_bf[:, 0:1], mulk[:],
                                         start=(k == 0), stop=(k == 1))
                    nc.scalar.copy(tg_sb[0:1, cs:cs + 128], pst)

                def emit_mms(psz, lhsTs, w_sb):
                    nk = len(lhsTs)
                    for k, lt in enumerate(lhsTs):
                        for nh in range(2):
                            nc.tensor.matmul(
                                psz[:, 512 * nh:512 * nh + 512],
                                lt,
                                w_sb[:, k, 512 * nh:512 * nh + 512],
                                start=(k == 0), stop=(k == nk - 1),
                            )

                def lstm_tail(psz, c_sb, bias_sb):
                    """Gate activations + cell update. Gate col order
                    [i, o, j, f]; the 0.5 sigmoid input scale for i/o/f is
                    folded into W on the host, so call 1 is a plain tanh
                    over [i,o,j] and call 2 a biased tanh over [f]
                    (sigmoid(x) = 0.5*tanh(x/2) + 0.5; the outer affine is
                    applied by affine_mul_reduce)."""
                    if bias_sb is not None:
                        nc.vector.tensor_tensor(
                            out=psz[:], in0=psz[:], in1=bias_sb[:],
                            op=ALU.add)
                    g = lw.tile([32, G4], BF16, tag="g")
                    nc.scalar.activation(g[:, 0:768], psz[:, 0:768], AF.Tanh)
                    nc.scalar.activation(g[:, 768:1024], psz[:, 768:1024],
                                         AF.Tanh, bias=half_sb[0:32, :1])
                    junk = lw.tile([32, 1], F32, tag="junk")
                    t1 = lw.tile([32, H], BF16, tag="t1")
                    nc.vector.affine_mul_reduce(
                        t1[:], junk[:], g[:, 0:256], g[:, 512:768], 0.5, 0.5)
                    cf = lw.tile([32, H], F32, tag="cf")
                    nc.vector.affine_mul_reduce(
                        cf[:], junk[:], g[:, 768:1024], c_sb[:], 0.5, 0.5)
                    nc.vector.tensor_tensor(out=c_sb[:], in0=cf[:],
                                            in1=t1[:], op=ALU.add)
                    tc_t = lw.tile([32, H], BF16, tag="tc")
                    nc.scalar.activation(tc_t[:], c_sb[:], AF.Tanh)
                    hrow = lw.tile([32, H], BF16, tag="hrow")
                    nc.vector.affine_mul_reduce(
                        hrow[:], junk[:], g[:, 256:512], tc_t[:], 0.5, 0.5)
                    return hrow

                def transpose_to(hrow, dst0, dst1):
                    """hrow [32,256] -> hidden-major k-tiles via DVE 32x32
                    StreamTranspose blocks (cross-partition block writes)."""
                    for q in range(4):
                        nc.vector.transpose(
                            dst0[32 * q:32 * q + 32],
                            hrow[:, 32 * q:32 * q + 32])
                        nc.vector.transpose(
                            dst1[32 * q:32 * q + 32],
                            hrow[:, 128 + 32 * q:128 + 32 * q + 32])

                # ---- wavefront: slot t = L1 step t  +  L2 step t-1 ----
                h1T_prev = None
                for t in range(T + 1):
                    # PE fill: one logits chunk-pair per slot (tile rows are
                    # complete 2 slots before first use)
                    ei = t - 6
                    if ei >= 0:
                        emit_logits_pair(ei // 4, ei % 4)

                    h1T_tm1 = h1T_prev
                    psz1 = psz2 = None
                    if t < T:
                        ts0 = 32 * t
                        psz1 = zp.tile([32, G4], F32, tag="z")
                        lhsTs = [xs0[:, ts0:ts0 + 32], xs1[:, ts0:ts0 + 32]]
                        if h1T_tm1 is not None:
                            lhsTs += [h1T_tm1[:, 0:32], h1T_tm1[:, 32:64]]
                        emit_mms(psz1, lhsTs, w1_sb)
                    if t >= 1:
                        tp0 = 32 * (t - 1)
                        psz2 = zp.tile([32, G4], F32, tag="z")
                        lhsTs2 = [h1T_tm1[:, 0:32], h1T_tm1[:, 32:64]]
                        if t >= 2:
                            tq0 = 32 * (t - 2)
                            lhsTs2 += [hs0[:, tq0:tq0 + 32],
                                       hs1[:, tq0:tq0 + 32]]
                        emit_mms(psz2, lhsTs2, w2_sb)

                    if psz1 is not None:
                        h1row = lstm_tail(psz1, c1,
                                          b1_sb if has_b1 else None)
                        h1T = lw.tile([128, 64], BF16, tag="h1T")
                        transpose_to(h1row, h1T[:, 0:32], h1T[:, 32:64])
                        h1T_prev = h1T
                    if psz2 is not None:
                        h2row = lstm_tail(psz2, c2,
                                          b2_sb if has_b2 else None)
                        tp0 = 32 * (t - 1)
                        transpose_to(h2row, hs0[:, tp0:tp0 + 32],
                                     hs1[:, tp0:tp0 + 32])

                # trailing logits pairs
                for ei in range(T - 5, RT * NPAIR):
                    emit_logits_pair(ei // 4, ei % 4)

            nc.sync.dma_start(se_d[:], se_sb[:])
            nc.sync.dma_start(tg_d[:], tg_sb[:])

    nc.compile()
    meta = dict(T=T, V=V, n_cores=n_cores, B=B, H=H, VS=VS, BT=BT, RT=RT,
                CH=CH, NCHUNK=NCHUNK, NPAIR=NPAIR)
    return nc, meta


# ---------------- host-side prep / combine ----------------

def prep_inputs(meta, input_data, targets, embedding, W1, b1, W2, b2,
                softmax_w, softmax_b):
    """Build the per-core input maps (numpy)."""
    B, T, V = meta["B"], meta["T"], meta["V"]
    VS, RT, n_cores = meta["VS"], meta["RT"], meta["n_cores"]
    H = meta["H"]
    G4 = 4 * H

    ids_tm = np.ascontiguousarray(
        np.asarray(input_data, np.int64).T).reshape(-1)
    tgt_tm = np.ascontiguousarray(
        np.asarray(targets, np.int64).T).reshape(-1)
    ids_in = ids_tm.astype(np.int32).reshape(RT, 128, 1)

    # W column permutation [i, j, f, o] (TF order) -> [i, o, j, f], with the
    # 0.5 sigmoid input scale folded into the i/o/f columns (the device does
    # a plain tanh over [i,o,j] and tanh(x + 0.5) over [f])
    perm = np.concatenate([
        np.arange(0, H), np.arange(3 * H, 4 * H),
        np.arange(H, 2 * H), np.arange(2 * H, 3 * H)])
    gate_scale = np.concatenate([
        np.full(2 * H, 0.5, np.float32),          # i, o
        np.ones(H, np.float32),                   # j
        np.full(H, 0.5, np.float32)])             # f

    def prep_w(W):
        Wp = (W[:, perm] * gate_scale[None, :]).astype(ml_dtypes.bfloat16)
        return np.ascontiguousarray(Wp.reshape(4, 128, G4))

    w1_in = prep_w(np.asarray(W1, np.float32))
    w2_in = prep_w(np.asarray(W2, np.float32))
    b1p = np.tile((np.asarray(b1, np.float32)[perm]
                   * gate_scale).reshape(1, G4), (32, 1))
    b2p = np.tile((np.asarray(b2, np.float32)[perm]
                   * gate_scale).reshape(1, G4), (32, 1))

    sw = np.asarray(softmax_w, np.float32)                  # [H, V]
    swb = np.asarray(softmax_b, np.float32)

    # vectorized ap_gather index layout: idx i lives at partition i%16,
    # column i//16, replicated per 16-partition group
    rtA = (np.arange(RT) * 128)[:, None, None]
    pA = (np.arange(128) % 16)[None, :, None]
    qA = (np.arange(8) * 16)[None, None, :]
    gat = rtA + qA + pA                                     # [RT, 128, 8]

    maps, masks = [], []
    for c in range(n_cores):
        shard = sw[:, c * VS:(c + 1) * VS].astype(ml_dtypes.bfloat16)
        sw_in = np.ascontiguousarray(shard.reshape(2, 128, VS))
        swi = sw_in.view(np.int16)
        swp_in = np.ascontiguousarray(
            np.stack([swi, swi], axis=-1))                  # [2,128,VS,2]

        tl = tgt_tm - c * VS
        inr = (tl >= 0) & (tl < VS)
        tlc = np.where(inr, tl, 0).astype(np.int16)
        tgi = tlc[gat]                                      # [RT, 128, 8]
        m = dict(ids=ids_in, emb=np.asarray(embedding, np.float32),
                 w1=w1_in, w2=w2_in, sw=sw_in, swp=swp_in, tgi=tgi)
        if np.any(b1p):
            m["b1p"] = b1p
        if np.any(b2p):
            m["b2p"] = b2p
        if np.any(swb):
            m["swbp"] = np.ascontiguousarray(
                np.tile(swb[c * VS:(c + 1) * VS].reshape(1, VS), (128, 1)))
        maps.append(m)
        masks.append(inr.astype(np.float32))
    return maps, masks, ids_tm, tgt_tm


def combine_outputs(meta, results, masks, tgt_tm, softmax_b):
    """results: list of per-core dicts with se_out [128, RT*NPAIR] and
    tg_out [1, BT]. Returns the scalar cost (np.float32)."""
    B, T, BT = meta["B"], meta["T"], meta["BT"]
    RT, NPAIR = meta["RT"], meta["NPAIR"]
    se_all = np.zeros(BT, np.float64)
    tg_all = np.zeros(BT, np.float64)
    for c, r in enumerate(results):
        se = np.asarray(r["se_out"], np.float64)  # [128, RT*NPAIR]
        se = se.reshape(128, RT, NPAIR).sum(-1)   # [128, RT]
        se_all += se.T.reshape(-1)                # row r = rt*128 + p
        tg_all += np.asarray(r["tg_out"], np.float64)[0] * masks[c]
    tg_all += np.asarray(softmax_b, np.float64)[tgt_tm]
    loss = np.log(se_all) - tg_all
    return np.float32(loss.sum() / B / T)


# ---------------- public entry point ----------------

_CACHE = {}
last_exec_time_ns = None
last_trace_path = None


def _get_built(has_b1, has_b2, has_swb):
    key = (has_b1, has_b2, has_swb)
    if key not in _CACHE:
        _CACHE[key] = build_charrnn(T=T, V=V, n_cores=NCORES,
                                    has_b1=has_b1, has_b2=has_b2,
                                    has_swb=has_swb, num_devices=NCORES)
    return _CACHE[key]


def kernel(input_data, targets, embedding, W1, b1, W2, b2,
           softmax_w, softmax_b, _trace=False):
    global last_exec_time_ns, last_trace_path
    has_b1 = bool(np.any(np.asarray(b1)))
    has_b2 = bool(np.any(np.asarray(b2)))
    has_swb = bool(np.any(np.asarray(softmax_b)))
    nc, meta = _get_built(has_b1, has_b2, has_swb)
    maps, masks, ids_tm, tgt_tm = prep_inputs(
        meta, input_data, targets, embedding, W1, b1, W2, b2,
        softmax_w, softmax_b)
    res = run_bass_kernel_spmd(nc, maps, core_ids=list(range(NCORES)),
                               trace=_trace)
    last_exec_time_ns = res.exec_time_ns
    if res.instructions_and_trace is not None:
        last_trace_path = res.instructions_and_trace[1]
    cost = combine_outputs(meta, res.results, masks, tgt_tm, softmax_b)
    return np.asarray(cost, np.float32)
